# revision 7
# baseline (speedup 1.0000x reference)
import threading
import numpy as np
import jax
import jax.numpy as jnp

# Problem constants (hardcoded per spec)
B, L, D, N = 4, 4096, 1024, 512
LN_EPS = 1e-5
CH = 128          # chunk length
NC = L // CH      # 32 chunks
NDEV = 8
DS = D // NDEV    # channels per device

_IDX = np.arange(CH)[:, None] - np.arange(CH)[None, :]
_MASK = (_IDX >= 0)[:, :, None]
_IDXC = np.clip(_IDX, 0, CH - 1)

_BF16 = jnp.bfloat16
_F32 = jnp.float32


def _dss_dev(q, p, m, lr, li, Ctr, Cti, g, bta, Dv):
    # q: [B,L,DS] int8   p,m: [B,L] f16 (rowscale*rstd, mu*rstd)
    # lr/li: [N] f32 (Lambda)   Ctr/i: [N,DS] f16   g,bta,Dv: [DS] f32
    u = q.astype(_F32) * p.astype(_F32)[..., None] - m.astype(_F32)[..., None]
    u = u * g + bta                               # [B,L,DS] f32
    ub = u.astype(_BF16)
    uc = ub.reshape(B, NC, CH, DS)

    # T[k,n] = exp(lam*k), k = 0..CH
    k = jnp.arange(CH + 1, dtype=_F32)[:, None]
    mag = jnp.exp(lr[None, :] * k)
    ph = li[None, :] * k
    Tr32 = mag * jnp.cos(ph)
    Ti32 = mag * jnp.sin(ph)
    EPr, EPi = Tr32[CH], Ti32[CH]                 # [N] f32
    Tr, Ti = Tr32.astype(_BF16), Ti32.astype(_BF16)

    A1r = jnp.flip(Tr[:CH], 0)                    # [s,n]: exp(lam*(CH-1-s))
    A1i = jnp.flip(Ti[:CH], 0)
    E2r = Tr[1:CH + 1]                            # [t,n]: exp(lam*(t+1))
    E2i = Ti[1:CH + 1]

    # local chunk states: Sloc[c,n,b,d] = sum_s A1[s,n] u[b,c,s,d]
    Slr = jnp.einsum('sn,bcsd->cnbd', A1r, uc, preferred_element_type=_F32)
    Sli = jnp.einsum('sn,bcsd->cnbd', A1i, uc, preferred_element_type=_F32)

    # scan over chunks: emitted state at step c covers chunks < c
    def step(carry, sl):
        sr, si = carry
        slr, sli = sl
        nsr = EPr[:, None, None] * sr - EPi[:, None, None] * si + slr
        nsi = EPr[:, None, None] * si + EPi[:, None, None] * sr + sli
        return (nsr, nsi), (sr, si)
    z = jnp.zeros((N, B, DS), _F32)
    _, (Spr, Spi) = jax.lax.scan(step, (z, z), (Slr, Sli))   # [NC,N,B,DS]

    Cr = Ctr.astype(_F32)[None, :, None, :]
    Ci = Cti.astype(_F32)[None, :, None, :]
    Wr = (Cr * Spr - Ci * Spi).astype(_BF16)
    Wi = (Cr * Spi + Ci * Spr).astype(_BF16)

    y_int = (jnp.einsum('tn,cnbd->bctd', E2r, Wr, preferred_element_type=_F32)
             - jnp.einsum('tn,cnbd->bctd', E2i, Wi, preferred_element_type=_F32))

    # intra-chunk kernel: Kl[tau,d] = Re sum_n T[tau,n] Ct[n,d]
    Kl = (jnp.einsum('tn,nd->td', Tr[:CH], Ctr.astype(_BF16), preferred_element_type=_F32)
          - jnp.einsum('tn,nd->td', Ti[:CH], Cti.astype(_BF16), preferred_element_type=_F32))
    Ttoe = jnp.where(_MASK, Kl.astype(_BF16)[_IDXC, :], 0)   # [t,s,d]
    y_intra = jnp.einsum('tsd,bcsd->bctd', Ttoe, uc, preferred_element_type=_F32)

    y = (y_int + y_intra).reshape(B, L, DS) + u * Dv
    yrm = jnp.maximum(jnp.max(jnp.abs(y), axis=-1), 1e-30)   # [B,L]
    qy = jnp.rint(y * (127.0 / yrm)[..., None]).astype(jnp.int8)
    return qy, (yrm * (1.0 / 127.0)).astype(jnp.float16)


_CACHE = {}


def _get_fn():
    if 'fn' not in _CACHE:
        _CACHE['fn'] = jax.jit(_dss_dev)
    return _CACHE['fn']


def kernel(x, Lambda_real, Lambda_imag, C_real, C_imag, param_D, ln_gamma, ln_beta):
    f16 = np.float16

    x32 = np.asarray(x, np.float32)

    # ---- LayerNorm stats + int8 row quantization on host ----
    s1 = x32.sum(-1)
    s2 = np.einsum('bld,bld->bl', x32, x32)
    mu = s1 / D
    var = s2 / D - mu * mu
    rstd = 1.0 / np.sqrt(var + LN_EPS)            # [B,L]
    rm = np.maximum(np.maximum(x32.max(-1), -x32.min(-1)), 1e-30)
    inv_scale = (127.0 / rm)[:, :, None]
    p16 = ((rm / 127.0) * rstd).astype(f16)
    m16 = (mu * rstd).astype(f16)

    fn = _get_fn()
    devs = jax.devices()[:NDEV]
    out = np.empty((B, L, D), np.float32)

    def fetch(i, fut):
        qy, rs = jax.device_get(fut)
        out[:, :, i * DS:(i + 1) * DS] = qy * rs.astype(np.float32)[:, :, None]

    # start the big transfer of chain 0 before computing the small tables
    q0 = np.rint(x32[:, :, :DS] * inv_scale).astype(np.int8)
    dq0 = jax.device_put(q0, devs[0])

    # ---- kernel params (host, cheap) ----
    lr64 = -np.exp(np.asarray(Lambda_real, np.float64))
    li64 = np.exp(np.asarray(Lambda_imag, np.float64))
    lam = lr64 + 1j * li64                                     # [N]
    Cc = np.asarray(C_real, np.float64) + 1j * np.asarray(C_imag, np.float64)
    Ct = (Cc * (np.exp(lam) - 1.0) / lam).T                    # [N,D]
    Ctr = np.real(Ct).astype(f16)
    Cti = np.imag(Ct).astype(f16)
    lr = lr64.astype(np.float32)
    li = li64.astype(np.float32)

    Dv = np.asarray(param_D, np.float32)
    g = np.asarray(ln_gamma, np.float32)
    bta = np.asarray(ln_beta, np.float32)

    threads = []
    for i, dev in enumerate(devs):
        sl = slice(i * DS, (i + 1) * DS)
        dq = dq0 if i == 0 else jax.device_put(
            np.rint(x32[:, :, sl] * inv_scale).astype(np.int8), dev)
        rest = [p16, m16, lr, li,
                np.ascontiguousarray(Ctr[:, sl]),
                np.ascontiguousarray(Cti[:, sl]),
                np.ascontiguousarray(g[sl]), np.ascontiguousarray(bta[sl]),
                np.ascontiguousarray(Dv[sl])]
        drest = jax.device_put(rest, [dev] * len(rest))
        fut = fn(dq, *drest)
        th = threading.Thread(target=fetch, args=(i, fut))
        th.start()
        threads.append(th)
    for th in threads:
        th.join()
    return out
